# revision 5
# baseline (speedup 1.0000x reference)
"""CLIP-style loss kernel for Trainium2 (8 NeuronCores, SPMD data-parallel).

Problem: two patch-embeddings (stride-4 4x4 conv -> L2 normalize) of
imgs/hha [32,64,128,128], per-sample logits = exp(logit_scale) * a @ h^T
[B,1024,1024], symmetric cross-entropy with diagonal labels, scalar loss.

Sharding: data-parallel over batch, 4 samples per core. Each core reads only
its 4x2 images; produces per-sample partial sums (row-sums, col-sums of
exp-logits, diag); host combines with float64 logs.

Per-core pipeline (v4 -- ACT-paced):
  The scalar (ACT) engine's 32 exps (~1.28us each) are the unavoidable
  critical mass (~41us), so the schedule is built around keeping ACT
  saturated from the earliest possible moment:
  - prologue: PE-warmup dummy matmuls (HAM un-throttle, 1.2->2.4 GHz)
    while the first image DMA lands (fine-grained per-(kc,t) chunks for
    sample 0 so conv starts ASAP); conv(0); tail(0).
  - loop b: consume(b) = per chunk k: L matmuls -> ONE exp (FD=1024,
    a-side row scale via ACT per-partition scale, row sums via accum_out
    straight into OUT) -> csum adds on DVE; side work (conv(b+1) chained
    with tail(b+1), fin(b-1)) drains in the exp gaps.
  - steady-state engine split per sample (~10.2us ACT pace): DVE keeps
    bias/cast + csum + t_ (~9us); GPSIMD takes sq + quake-rsqrt + h_hat
    muls for b>=1 (latency off DVE); PE does conv/n2c/bc/L/fin reduces
    (~6.5us, stays HAM-warm since L matmuls recur every ~1.3us).
  - no final Ln on device: raw row/col exp-sums and diag partials are
    DMA'd out; the host does log() in float64 (removes the 2nd ACT table
    load and the end-of-kernel ACT serialization).
Output per core: [128, 68] partial-sum block; host reduces.
"""

import os
import sys
from contextlib import ExitStack
from itertools import chain

import numpy as np

for _p in ("/opt/trn_rl_repo", "/root/.axon_site/_ro/trn_rl_repo"):
    if os.path.isdir(_p) and _p not in sys.path:
        sys.path.insert(0, _p)

import concourse.bass as bass
import concourse.mybir as mybir
import concourse.tile as tile
from concourse import bacc
from concourse.bass_utils import run_bass_kernel_spmd

F32 = mybir.dt.float32
I32 = mybir.dt.int32
BF16 = mybir.dt.bfloat16
FP8 = mybir.dt.float8e4
AF = mybir.ActivationFunctionType
ALU = mybir.AluOpType
DR = mybir.MatmulPerfMode.DoubleRow

N_CORES = 8
B_FULL = 32
BPC = B_FULL // N_CORES  # samples per core
C, H, W, D, P = 64, 128, 128, 128, 4
NPAT = (H // P) * (W // P)  # 1024 patches
NH = NPAT // 2  # 512 (one patch-half / one PSUM bank)
NOFF = P * P  # 16 kernel offsets
NCHUNK = NPAT // 128  # 8 logit row chunks
NKC2 = (C * NOFF) // 256  # 4 conv contraction chunks (K=256, DoubleRow)

QUAKE_C = 1597463007.0  # 0x5f3759df as a float value
WSCALE = 64.0  # fp8 weight pre-scale (cancels through normalization)
N_WU = 18  # PE warm-up dummy matmuls (HAM un-throttle during first DMA)

# OUT layout: [128, 2*NCHUNK*BPC + BPC] = [128, 68]
#   cols NCHUNK*b + k        : row sums of exp chunk (b,k)    (ACT accum)
#   cols 32 + NCHUNK*b + k   : col sums T-layout per sample   (DVE copy)
#   cols 64 + b              : diag partial sums [128,1]      (DVE reduce)
NOUT = 2 * NCHUNK * BPC + BPC


def build_program(ln_s: float) -> bass.Bass:
    nc = bacc.Bacc(None)
    s2inv = float(np.exp(-2.0 * ln_s))  # 1/s^2

    imgs = nc.declare_dram_parameter(
        "imgs", [BPC, 128, NKC2, 2, NPAT], FP8, isOutput=False
    )
    hha = nc.declare_dram_parameter(
        "hha", [BPC, 128, NKC2, 2, NPAT], FP8, isOutput=False
    )
    w1t = nc.declare_dram_parameter("w1t", [128, NKC2, 2, D], FP8, isOutput=False)
    w2t = nc.declare_dram_parameter("w2t", [128, NKC2, 2, D], FP8, isOutput=False)
    b1 = nc.declare_dram_parameter("b1", [D], F32, isOutput=False)
    b2 = nc.declare_dram_parameter("b2", [D], F32, isOutput=False)
    ident_d = nc.declare_dram_parameter("ident", [128, 128], BF16, isOutput=False)
    sel_d = nc.declare_dram_parameter("sel", [8, NCHUNK * 128], BF16, isOutput=False)
    out_d = nc.declare_dram_parameter("out", [128, NOUT], F32, isOutput=True)

    srcs = (imgs, hha)

    with tile.TileContext(nc) as tc, ExitStack() as ctx:
        # SBUF pools
        p_img = ctx.enter_context(tc.tile_pool(name="img", bufs=4))
        p_one = ctx.enter_context(tc.tile_pool(name="singles", bufs=1))
        p_ysb = ctx.enter_context(tc.tile_pool(name="ysb", bufs=6))
        p_sq = ctx.enter_context(tc.tile_pool(name="sq", bufs=4))
        p_hhat = ctx.enter_context(tc.tile_pool(name="hhat", bufs=2))
        p_E = ctx.enter_context(tc.tile_pool(name="E", bufs=6))
        p_cs = ctx.enter_context(tc.tile_pool(name="cs", bufs=2))
        p_sm = ctx.enter_context(tc.tile_pool(name="small", bufs=2))
        p_n2 = ctx.enter_context(tc.tile_pool(name="n2", bufs=3))
        # PSUM pools (8 banks: conv/bc 2x1 + logits 2x2 + T 2x1)
        pp_c = ctx.enter_context(tc.tile_pool(name="ppc", bufs=2, space="PSUM"))
        pp_L = ctx.enter_context(tc.tile_pool(name="ppL", bufs=2, space="PSUM"))
        pp_T = ctx.enter_context(tc.tile_pool(name="ppT", bufs=2, space="PSUM"))

        # PE warm-up scratch: memset is the first vector instruction, so PE
        # can start issuing dummy matmuls as soon as programs are fetched.
        scratch = p_one.tile([128, 256], BF16)
        nc.vector.memset(scratch, 0.25)

        # weights first on the sync queue (the first conv waits on them);
        # small constants go via SWDGE (gpsimd) to keep sync free for images
        wts = []
        biases = []
        for wsrc, bsrc in ((w1t, b1), (w2t, b2)):
            wt = p_one.tile([128, NKC2, 2, D], FP8, tag=f"wt_{wsrc.name}")
            nc.sync.dma_start(out=wt, in_=wsrc[:])
            wts.append(wt)
            bt = p_one.tile([128, 1], F32, tag=f"bias_{bsrc.name}")
            nc.gpsimd.dma_start(out=bt, in_=bsrc[:].rearrange("(d one) -> d one", one=1))
            biases.append(bt)
        ones_k = p_one.tile([128, 1], BF16)
        nc.vector.memset(ones_k, 1.0)
        ident = p_one.tile([128, 128], BF16)
        nc.gpsimd.dma_start(out=ident, in_=ident_d[:])
        sel = p_one.tile([8, NCHUNK * 128], BF16)
        nc.gpsimd.dma_start(out=sel, in_=sel_d[:])
        OUT = p_one.tile([128, NOUT], F32)
        # scm: quake input scale (col 0-7: 1/s^2 for the a-side, 8-15: 1.0)
        scm = p_one.tile([128, 16], F32)
        nc.vector.memset(scm[:, 0:8], s2inv)
        nc.vector.memset(scm[:, 8:16], 1.0)
        # persistent per-sample [invT | invh] blocks
        invTH = p_one.tile([128, 16 * BPC], F32)

        # HAM warm-up: keep PE busy from program-fetch until the first image
        # chunk lands, so convs run at 2.4 GHz instead of 1.2.
        wu = pp_T.tile([128, 256], F32, tag="T", name="wu")
        for _ in range(N_WU):
            nc.tensor.matmul(
                wu, scratch[:, 0:128], scratch, start=True, stop=True
            )

        def conv_work(b, out):
            """Generator: load + conv + bias + sq + T-norm^2 for sample b.
            Yields between quanta so the driver can interleave emission.
            For b>0, sq runs on GPSIMD (DVE budget); b==0 stays on DVE for
            latency (it gates the first exp)."""
            sq_eng = nc.vector if b == 0 else nc.gpsimd
            y_sb = {}
            n2c = pp_T.tile([128, 16], F32, tag="T", name=f"n2c_{b}")
            for m in range(2):
                img = p_img.tile([128, NKC2, 2, NPAT], FP8, tag="img")
                if b == 0:
                    # fine-grained: per-(patch-half, kc) chunks, half 0
                    # first, so the first conv group starts ASAP
                    for t in range(2):
                        for kc in range(NKC2):
                            nc.sync.dma_start(
                                out=img[:, kc, :, t * NH : (t + 1) * NH],
                                in_=srcs[m][b][:, kc, :, t * NH : (t + 1) * NH],
                            )
                else:
                    nc.sync.dma_start(out=img, in_=srcs[m][b])

                ym = p_ysb.tile([128, NPAT], BF16, tag="ysb")
                sq = p_sq.tile([128, NPAT], BF16, tag="sq")
                for t in range(2):
                    Y = pp_c.tile([128, NH], F32, tag="c", name=f"cv_{b}_{m}_{t}")
                    for kc in range(NKC2):
                        nc.tensor.matmul(
                            Y,
                            wts[m][:, kc],
                            img[:, kc, :, t * NH : (t + 1) * NH],
                            start=(kc == 0),
                            stop=(kc == NKC2 - 1),
                            perf_mode=DR,
                        )
                    nc.vector.tensor_scalar_add(
                        ym[:, t * NH : (t + 1) * NH], Y, biases[m]
                    )
                    sq_eng.tensor_mul(
                        sq[:, t * NH : (t + 1) * NH],
                        ym[:, t * NH : (t + 1) * NH],
                        ym[:, t * NH : (t + 1) * NH],
                    )
                    yield
                y_sb[m] = ym
                for k0 in range(0, NCHUNK, 4):
                    for k in range(k0, k0 + 4):
                        nc.tensor.matmul(
                            n2c[:, m * 8 + k : m * 8 + k + 1],
                            sq[:, 128 * k : 128 * (k + 1)],
                            ones_k,
                            start=True,
                            stop=True,
                        )
                    yield
            n2s = p_n2.tile([128, 16], F32, tag="n2")
            nc.vector.tensor_copy(n2s, n2c)  # free the PSUM bank early
            out["y"] = y_sb
            out["n2"] = n2s

        def tail_work(b, cs, out):
            """Generator: rsqrt + h_hat for sample b. For b>0, the quake
            chain and h_hat muls run on GPSIMD to keep DVE under the ACT
            pace; b==0 stays on DVE for latency."""
            ve = nc.vector if b == 0 else nc.gpsimd
            n2s, y_sb = cs["n2"], cs["y"]
            # quake rsqrt on [128,16] (cols 0-7: a-side with s folded; 8-15: h)
            qx = p_sm.tile([128, 16], F32, tag="qx")
            ve.tensor_mul(qx, n2s, scm)
            qf = p_sm.tile([128, 16], F32, tag="qf")
            ve.tensor_copy(qf, qx[:].bitcast(I32))  # int-value as float
            qi = p_sm.tile([128, 16], I32, tag="qi")
            ve.tensor_scalar(
                qi, qf, -0.5, QUAKE_C, op0=ALU.mult, op1=ALU.add
            )
            yield
            y0 = qi[:].bitcast(F32)
            qt = p_sm.tile([128, 16], F32, tag="qt")
            ve.tensor_mul(qt, y0, y0)
            ve.tensor_mul(qt, qt, qx)
            ve.tensor_scalar(qt, qt, -0.5, 1.5, op0=ALU.mult, op1=ALU.add)
            inv = invTH[:, 16 * b : 16 * (b + 1)]
            ve.tensor_mul(inv, y0, qt)
            yield
            # h_hat: invh -> bf16 -> PE transpose -> selector broadcast -> mul
            ihb = p_sm.tile([128, 8], BF16, tag="ihb")
            ve.tensor_copy(ihb, inv[:, 8:16])
            ih8 = pp_T.tile([8, 128], BF16, tag="T", name=f"ih8_{b}")
            nc.tensor.transpose(ih8, ihb, ident)
            # PSUM reads must stay off GPSIMD (no PSUM port)
            ih8s = p_sm.tile([8, 128], BF16, tag="ih8s")
            nc.vector.tensor_copy(ih8s, ih8)
            yield
            h_hat = p_hhat.tile([128, NPAT], BF16, tag="hhat")
            for t in range(2):
                bc = pp_c.tile([128, NH], F32, tag="c", name=f"bc_{b}_{t}")
                for q in range(4):
                    qq = 4 * t + q
                    nc.tensor.matmul(
                        bc[:, 128 * q : 128 * (q + 1)],
                        sel[:, 128 * qq : 128 * (qq + 1)],
                        ih8s,
                        start=True,
                        stop=True,
                    )
                nc.vector.tensor_mul(
                    h_hat[:, t * NH : (t + 1) * NH],
                    y_sb[1][:, t * NH : (t + 1) * NH],
                    bc,
                )
                yield
            out["y"] = y_sb
            out["h"] = h_hat

        def fin_work(b, st):
            """Generator: T-layout partial sums for sample b (col sums +
            diag); runs inside iteration b+1's exp window."""
            csum, t_ = st["cs"], st["t"]
            ct16 = pp_T.tile([128, 16], F32, tag="T", name=f"ct16_{b}")
            for k0 in range(0, NCHUNK, 4):
                for k in range(k0, k0 + 4):
                    nc.tensor.matmul(
                        ct16[:, 8 + k : 9 + k],
                        t_[:, 128 * k : 128 * (k + 1)],
                        ones_k,
                        start=True,
                        stop=True,
                    )
                yield
            for k0 in range(0, NCHUNK, 4):
                for k in range(k0, k0 + 4):
                    nc.tensor.matmul(
                        ct16[:, k : k + 1],
                        csum[:, 128 * k : 128 * (k + 1)],
                        ones_k,
                        start=True,
                        stop=True,
                    )
                yield
            base = NCHUNK * (BPC + b)
            nc.vector.tensor_copy(OUT[:, base : base + NCHUNK], ct16[:, 0:8])
            dg = p_sm.tile([128, NCHUNK], F32, tag="dg")
            nc.vector.tensor_mul(
                dg, ct16[:, 8:16], invTH[:, 16 * b : 16 * b + 8]
            )
            nc.vector.tensor_reduce(
                out=OUT[:, 2 * NCHUNK * BPC + b : 2 * NCHUNK * BPC + b + 1],
                in_=dg,
                axis=mybir.AxisListType.X,
                op=ALU.add,
            )

        def drain(gens, n=1):
            """Advance each live generator up to n quanta."""
            for g in list(gens):
                for _ in range(n):
                    try:
                        next(g)
                    except StopIteration:
                        gens.remove(g)
                        break

        def consume(b, st, side):
            """Logits + exp + csum for sample b, interleaving side work in
            the exp gaps."""
            t_ = p_sq.tile([128, NPAT], BF16, tag="sq")
            nc.vector.tensor_mul(t_, st["y"][0], st["h"])

            csum = p_cs.tile([128, NPAT], BF16, tag="cs")
            Es = {}
            ya, h_hat = st["y"][0], st["h"]
            for k in range(NCHUNK):
                L = pp_L.tile([128, NPAT], F32, tag="L", name=f"L_{b}_{k}")
                for j in range(2):
                    nc.tensor.matmul(
                        L[:, j * NH : (j + 1) * NH],
                        ya[:, 128 * k : 128 * (k + 1)],
                        h_hat[:, j * NH : (j + 1) * NH],
                        start=True,
                        stop=True,
                    )
                E = p_E.tile([128, NPAT], BF16, tag="E", name=f"E_{b}_{k}")
                Es[k] = E
                nc.scalar.activation(
                    out=E,
                    in_=L,
                    func=AF.Exp,
                    scale=invTH[:, 16 * b + k : 16 * b + k + 1],
                    accum_out=OUT[:, NCHUNK * b + k : NCHUNK * b + k + 1],
                )
                if k == 1:
                    nc.vector.tensor_add(csum, Es[0], Es[1])
                elif k > 1:
                    nc.vector.tensor_add(csum, csum, E)
                drain(side, 1)
            return {"cs": csum, "t": t_}

        # ACT-paced interleaved pipeline
        outs = {b: {} for b in range(BPC)}
        tout = {b: {} for b in range(BPC)}
        for _ in conv_work(0, outs[0]):
            pass
        for _ in tail_work(0, outs[0], tout[0]):
            pass
        fin_prev = None
        for b in range(BPC):
            side = []
            if b + 1 < BPC:
                side.append(
                    chain(
                        conv_work(b + 1, outs[b + 1]),
                        tail_work(b + 1, outs[b + 1], tout[b + 1]),
                    )
                )
            if fin_prev is not None:
                side.append(fin_prev)
            st = consume(b, tout[b], side)
            drain(side, 100)  # finish any leftovers
            fin_prev = fin_work(b, st)
        for _ in fin_prev:
            pass

        nc.sync.dma_start(out=out_d[:], in_=OUT)

    nc.compile()
    return nc


_PROGRAM_CACHE: dict = {}


def _get_program(ln_s: float) -> bass.Bass:
    key = round(float(ln_s), 9)
    if key not in _PROGRAM_CACHE:
        _PROGRAM_CACHE[key] = build_program(float(ln_s))
    return _PROGRAM_CACHE[key]


def make_in_maps(imgs, hha, w1, b1, w2, b2):
    """Shard full inputs into per-core input maps (host-side, cheap)."""
    import ml_dtypes

    bf16 = ml_dtypes.bfloat16
    fp8 = ml_dtypes.float8_e4m3

    def prep_w(w):
        # [D,C,P,P] -> [(c,di,dj)=1024, D] -> [feat%128, chunk, ko, D] fp8 x64
        wf = np.transpose(np.asarray(w), (1, 2, 3, 0)).reshape(C * NOFF, D)
        wf = np.clip(wf * WSCALE, -240.0, 240.0)
        return np.ascontiguousarray(
            wf.reshape(NKC2, 2, 128, D).transpose(2, 0, 1, 3)
        ).astype(fp8)

    def prep_x(x):
        # stride==kernel -> im2col is a permutation:
        # [B,C,H,W] -> [B, (c,di,dj)=1024, (i,j)=1024] -> [B,128,NKC2,2,NPAT]
        B = x.shape[0]
        xp = np.asarray(x).reshape(B, C, H // P, P, W // P, P)
        xp = xp.transpose(0, 1, 3, 5, 2, 4).reshape(B, C * NOFF, NPAT)
        xp = np.clip(xp, -240.0, 240.0)
        return np.ascontiguousarray(
            xp.reshape(B, NKC2, 2, 128, NPAT).transpose(0, 3, 1, 2, 4)
        ).astype(fp8)

    w1t = prep_w(w1)
    w2t = prep_w(w2)
    imgs = prep_x(imgs)
    hha = prep_x(hha)
    b1 = np.ascontiguousarray(np.asarray(b1) * WSCALE, dtype=np.float32)
    b2 = np.ascontiguousarray(np.asarray(b2) * WSCALE, dtype=np.float32)
    ident = np.eye(128, dtype=bf16)
    sel = np.zeros((8, NCHUNK * 128), dtype=bf16)
    for q in range(NCHUNK):
        sel[q, 128 * q : 128 * (q + 1)] = 1.0
    maps = []
    for i in range(N_CORES):
        maps.append(
            {
                "imgs": np.ascontiguousarray(imgs[i * BPC : (i + 1) * BPC]),
                "hha": np.ascontiguousarray(hha[i * BPC : (i + 1) * BPC]),
                "w1t": w1t,
                "w2t": w2t,
                "b1": b1,
                "b2": b2,
                "ident": ident,
                "sel": sel,
            }
        )
    return maps


def combine_outputs(outs) -> np.float32:
    """Reduce the 8 per-core [128, 68] partial blocks to the scalar loss.
    Cols 0:64 are raw row/col exp-sums (host takes log in f64); cols 64:68
    are per-sample diag partial sums."""
    tot = np.float64(0.0)
    for o in outs:
        o = np.asarray(o, dtype=np.float64)
        lse_rc = np.log(o[:, : 2 * NCHUNK * BPC]).sum()
        diag = o[:, 2 * NCHUNK * BPC :].sum()
        tot += 0.5 * lse_rc - diag
    return np.float32(tot / (B_FULL * NPAT))


def run_spmd(imgs, hha, w1, b1, w2, b2, logit_scale, **kwargs):
    """Run on the 8 cores; returns (loss, BassKernelResults)."""
    ln_s = float(np.asarray(logit_scale))
    nc = _get_program(ln_s)
    in_maps = make_in_maps(imgs, hha, w1, b1, w2, b2)
    res = run_bass_kernel_spmd(nc, in_maps, list(range(N_CORES)), **kwargs)
    return combine_outputs([r["out"] for r in res.results]), res


def kernel(imgs, hha, w1, b1, w2, b2, logit_scale):
    loss, _ = run_spmd(imgs, hha, w1, b1, w2, b2, logit_scale)
    if not np.isfinite(loss):  # one-shot retry on a transient device glitch
        loss, _ = run_spmd(imgs, hha, w1, b1, w2, b2, logit_scale)
    return loss


if __name__ == "__main__":
    # smoke test against a tiny numpy reference of the math
    rng = np.random.default_rng(0)
    imgs = rng.standard_normal((B_FULL, C, H, W), dtype=np.float32)
    hha = rng.standard_normal((B_FULL, C, H, W), dtype=np.float32)
    w1 = rng.standard_normal((D, C, P, P), dtype=np.float32) * 0.03
    w2 = rng.standard_normal((D, C, P, P), dtype=np.float32) * 0.03
    b1 = np.zeros(D, np.float32)
    b2 = np.zeros(D, np.float32)
    ls = np.float32(np.log(1.0 / 0.07))
    print(kernel(imgs, hha, w1, b1, w2, b2, ls))


# revision 8
# speedup vs baseline: 1.0305x; 1.0305x over previous
"""CLIP-style loss kernel for Trainium2 (8 NeuronCores, SPMD data-parallel).

Problem: two patch-embeddings (stride-4 4x4 conv -> L2 normalize) of
imgs/hha [32,64,128,128], per-sample logits = exp(logit_scale) * a @ h^T
[B,1024,1024], symmetric cross-entropy with diagonal labels, scalar loss.

Sharding: data-parallel over batch, 4 samples per core. Each core reads only
its 4x2 images; produces per-sample partial sums (row-sums, col-sums of
exp-logits, diag); host combines with float64 logs.

Per-core pipeline (v4 -- ACT-paced):
  The scalar (ACT) engine's 32 exps (~1.28us each) are the unavoidable
  critical mass (~41us), so the schedule is built around keeping ACT
  saturated from the earliest possible moment:
  - prologue: PE-warmup dummy matmuls (HAM un-throttle, 1.2->2.4 GHz)
    while the first image DMA lands (fine-grained per-(kc,t) chunks for
    sample 0 so conv starts ASAP); conv(0); tail(0).
  - loop b: consume(b) = per chunk k: L matmuls -> ONE exp (FD=1024,
    a-side row scale via ACT per-partition scale, row sums via accum_out
    straight into OUT) -> csum adds on DVE; side work (conv(b+1) chained
    with tail(b+1), fin(b-1)) drains in the exp gaps.
  - steady-state engine split per sample (~10.2us ACT pace): DVE keeps
    bias/cast + csum + t_ (~9us); GPSIMD takes sq + quake-rsqrt + h_hat
    muls for b>=1 (latency off DVE); PE does conv/n2c/bc/L/fin reduces
    (~6.5us, stays HAM-warm since L matmuls recur every ~1.3us).
  - no final Ln on device: raw row/col exp-sums and diag partials are
    DMA'd out; the host does log() in float64 (removes the 2nd ACT table
    load and the end-of-kernel ACT serialization).
Output per core: [128, 68] partial-sum block; host reduces.
"""

import os
import sys
from contextlib import ExitStack
from itertools import chain

import numpy as np

for _p in ("/opt/trn_rl_repo", "/root/.axon_site/_ro/trn_rl_repo"):
    if os.path.isdir(_p) and _p not in sys.path:
        sys.path.insert(0, _p)

import concourse.bass as bass
import concourse.mybir as mybir
import concourse.tile as tile
from concourse import bacc
from concourse.bass_utils import run_bass_kernel_spmd

F32 = mybir.dt.float32
I32 = mybir.dt.int32
BF16 = mybir.dt.bfloat16
FP8 = mybir.dt.float8e4
AF = mybir.ActivationFunctionType
ALU = mybir.AluOpType
DR = mybir.MatmulPerfMode.DoubleRow

N_CORES = 8
B_FULL = 32
BPC = B_FULL // N_CORES  # samples per core
C, H, W, D, P = 64, 128, 128, 128, 4
NPAT = (H // P) * (W // P)  # 1024 patches
NH = NPAT // 2  # 512 (one patch-half / one PSUM bank)
NOFF = P * P  # 16 kernel offsets
NCHUNK = NPAT // 128  # 8 logit row chunks
NKC2 = (C * NOFF) // 256  # 4 conv contraction chunks (K=256, DoubleRow)

QUAKE_C = 1597463007.0  # 0x5f3759df as a float value
WSCALE = 64.0  # fp8 weight pre-scale (cancels through normalization)
N_WU = 18  # PE warm-up dummy matmuls (HAM un-throttle during first DMA)

# OUT layout: [128, 2*NCHUNK*BPC + BPC] = [128, 68]
#   cols NCHUNK*b + k        : row sums of exp chunk (b,k)    (ACT accum)
#   cols 32 + NCHUNK*b + k   : col sums T-layout per sample   (DVE copy)
#   cols 64 + b              : diag partial sums [128,1]      (DVE reduce)
NOUT = 2 * NCHUNK * BPC + BPC


def build_program(ln_s: float) -> bass.Bass:
    nc = bacc.Bacc(None)
    s2inv = float(np.exp(-2.0 * ln_s))  # 1/s^2

    imgs = nc.declare_dram_parameter(
        "imgs", [BPC, 128, NKC2, 2, NPAT], FP8, isOutput=False
    )
    hha = nc.declare_dram_parameter(
        "hha", [BPC, 128, NKC2, 2, NPAT], FP8, isOutput=False
    )
    w1t = nc.declare_dram_parameter("w1t", [128, NKC2, 2, D], FP8, isOutput=False)
    w2t = nc.declare_dram_parameter("w2t", [128, NKC2, 2, D], FP8, isOutput=False)
    b1 = nc.declare_dram_parameter("b1", [D], F32, isOutput=False)
    b2 = nc.declare_dram_parameter("b2", [D], F32, isOutput=False)
    ident_d = nc.declare_dram_parameter("ident", [128, 128], BF16, isOutput=False)
    sel_d = nc.declare_dram_parameter("sel", [8, NCHUNK * 128], BF16, isOutput=False)
    out_d = nc.declare_dram_parameter("out", [128, NOUT], F32, isOutput=True)

    srcs = (imgs, hha)

    with tile.TileContext(nc) as tc, ExitStack() as ctx:
        # SBUF pools
        p_img = ctx.enter_context(tc.tile_pool(name="img", bufs=4))
        p_one = ctx.enter_context(tc.tile_pool(name="singles", bufs=1))
        p_ysb = ctx.enter_context(tc.tile_pool(name="ysb", bufs=6))
        p_sq = ctx.enter_context(tc.tile_pool(name="sq", bufs=4))
        p_hhat = ctx.enter_context(tc.tile_pool(name="hhat", bufs=2))
        p_E = ctx.enter_context(tc.tile_pool(name="E", bufs=6))
        p_cs = ctx.enter_context(tc.tile_pool(name="cs", bufs=2))
        p_sm = ctx.enter_context(tc.tile_pool(name="small", bufs=2))
        p_n2 = ctx.enter_context(tc.tile_pool(name="n2", bufs=3))
        # PSUM pools (8 banks: conv/bc 2x1 + logits 2x2 + T 2x1)
        pp_c = ctx.enter_context(tc.tile_pool(name="ppc", bufs=2, space="PSUM"))
        pp_L = ctx.enter_context(tc.tile_pool(name="ppL", bufs=2, space="PSUM"))
        pp_T = ctx.enter_context(tc.tile_pool(name="ppT", bufs=2, space="PSUM"))

        # PE warm-up scratch: memset is the first vector instruction, so PE
        # can start issuing dummy matmuls as soon as programs are fetched.
        scratch = p_one.tile([128, 256], BF16)
        nc.vector.memset(scratch, 0.25)

        # weights first on the sync queue (the first conv waits on them);
        # small constants go via SWDGE (gpsimd) to keep sync free for images
        wts = []
        biases = []
        for wsrc, bsrc in ((w1t, b1), (w2t, b2)):
            wt = p_one.tile([128, NKC2, 2, D], FP8, tag=f"wt_{wsrc.name}")
            nc.sync.dma_start(out=wt, in_=wsrc[:])
            wts.append(wt)
            bt = p_one.tile([128, 1], F32, tag=f"bias_{bsrc.name}")
            nc.gpsimd.dma_start(out=bt, in_=bsrc[:].rearrange("(d one) -> d one", one=1))
            biases.append(bt)
        ones_k = p_one.tile([128, 1], BF16)
        nc.vector.memset(ones_k, 1.0)
        ident = p_one.tile([128, 128], BF16)
        nc.gpsimd.dma_start(out=ident, in_=ident_d[:])
        sel = p_one.tile([8, NCHUNK * 128], BF16)
        nc.gpsimd.dma_start(out=sel, in_=sel_d[:])
        OUT = p_one.tile([128, NOUT], F32)
        # scm: quake input scale (col 0-7: 1/s^2 for the a-side, 8-15: 1.0)
        scm = p_one.tile([128, 16], F32)
        nc.vector.memset(scm[:, 0:8], s2inv)
        nc.vector.memset(scm[:, 8:16], 1.0)
        # persistent per-sample [invT | invh] blocks
        invTH = p_one.tile([128, 16 * BPC], F32)

        # HAM warm-up: keep PE busy from program-fetch until the first image
        # chunk lands, so convs run at 2.4 GHz instead of 1.2.
        wu = pp_T.tile([128, 256], F32, tag="T", name="wu")
        for _ in range(N_WU):
            nc.tensor.matmul(
                wu, scratch[:, 0:128], scratch, start=True, stop=True
            )

        def conv_dma(b):
            """Issue the image DMAs for sample b; returns the img tiles.
            Sample 0's m0 goes in two patch-half pieces so the first conv
            group can start while the second half streams."""
            tiles = []
            for m in range(2):
                img = p_img.tile([128, NKC2, 2, NPAT], FP8, tag="img")
                if b == 0 and m == 0:
                    for t in range(2):
                        nc.sync.dma_start(
                            out=img[:, :, :, t * NH : (t + 1) * NH],
                            in_=srcs[m][b][:, :, :, t * NH : (t + 1) * NH],
                        )
                else:
                    nc.sync.dma_start(out=img, in_=srcs[m][b])
                tiles.append(img)
            return tiles

        def conv_compute(b, imgs_t, out):
            """Generator: conv + bias + sq + T-norm^2 for sample b.
            For b>0, sq runs on GPSIMD (DVE budget); b==0 stays on DVE for
            latency (it gates the first exp)."""
            sq_eng = nc.vector if b == 0 else nc.gpsimd
            y_sb = {}
            n2c = pp_T.tile([128, 16], F32, tag="T", name=f"n2c_{b}")
            for m in range(2):
                img = imgs_t[m]
                ym = p_ysb.tile([128, NPAT], BF16, tag="ysb")
                sq = p_sq.tile([128, NPAT], BF16, tag="sq")
                for t in range(2):
                    Y = pp_c.tile([128, NH], F32, tag="c", name=f"cv_{b}_{m}_{t}")
                    for kc in range(NKC2):
                        nc.tensor.matmul(
                            Y,
                            wts[m][:, kc],
                            img[:, kc, :, t * NH : (t + 1) * NH],
                            start=(kc == 0),
                            stop=(kc == NKC2 - 1),
                            perf_mode=DR,
                        )
                    nc.vector.tensor_scalar_add(
                        ym[:, t * NH : (t + 1) * NH], Y, biases[m]
                    )
                    sq_eng.tensor_mul(
                        sq[:, t * NH : (t + 1) * NH],
                        ym[:, t * NH : (t + 1) * NH],
                        ym[:, t * NH : (t + 1) * NH],
                    )
                    if t == 0:
                        yield
                y_sb[m] = ym
                for k0 in range(0, NCHUNK, 4):
                    for k in range(k0, k0 + 4):
                        nc.tensor.matmul(
                            n2c[:, m * 8 + k : m * 8 + k + 1],
                            sq[:, 128 * k : 128 * (k + 1)],
                            ones_k,
                            start=True,
                            stop=True,
                        )
                    if k0 == 0:
                        yield
                yield
            n2s = p_n2.tile([128, 16], F32, tag="n2")
            nc.vector.tensor_copy(n2s, n2c)  # free the PSUM bank early
            out["y"] = y_sb
            out["n2"] = n2s



        def tail_work(b, cs, out):
            """Generator: rsqrt + h_hat for sample b. For b>0, the quake
            chain and h_hat muls run on GPSIMD to keep DVE under the ACT
            pace; b==0 stays on DVE for latency."""
            ve = nc.vector if b == 0 else nc.gpsimd
            n2s, y_sb = cs["n2"], cs["y"]
            # quake rsqrt on [128,16] (cols 0-7: a-side with s folded; 8-15: h)
            qx = p_sm.tile([128, 16], F32, tag="qx")
            ve.tensor_mul(qx, n2s, scm)
            qf = p_sm.tile([128, 16], F32, tag="qf")
            ve.tensor_copy(qf, qx[:].bitcast(I32))  # int-value as float
            qi = p_sm.tile([128, 16], I32, tag="qi")
            ve.tensor_scalar(
                qi, qf, -0.5, QUAKE_C, op0=ALU.mult, op1=ALU.add
            )
            yield
            y0 = qi[:].bitcast(F32)
            qt = p_sm.tile([128, 16], F32, tag="qt")
            ve.tensor_mul(qt, y0, y0)
            ve.tensor_mul(qt, qt, qx)
            ve.tensor_scalar(qt, qt, -0.5, 1.5, op0=ALU.mult, op1=ALU.add)
            inv = invTH[:, 16 * b : 16 * (b + 1)]
            ve.tensor_mul(inv, y0, qt)
            yield
            # h_hat: invh -> bf16 -> PE transpose -> selector broadcast -> mul
            ihb = p_sm.tile([128, 8], BF16, tag="ihb")
            ve.tensor_copy(ihb, inv[:, 8:16])
            ih8 = pp_T.tile([8, 128], BF16, tag="T", name=f"ih8_{b}")
            nc.tensor.transpose(ih8, ihb, ident)
            # PSUM reads must stay off GPSIMD (no PSUM port)
            ih8s = p_sm.tile([8, 128], BF16, tag="ih8s")
            nc.vector.tensor_copy(ih8s, ih8)
            yield
            h_hat = p_hhat.tile([128, NPAT], BF16, tag="hhat")
            for t in range(2):
                bc = pp_c.tile([128, NH], F32, tag="c", name=f"bc_{b}_{t}")
                for q in range(4):
                    qq = 4 * t + q
                    nc.tensor.matmul(
                        bc[:, 128 * q : 128 * (q + 1)],
                        sel[:, 128 * qq : 128 * (qq + 1)],
                        ih8s,
                        start=True,
                        stop=True,
                    )
                nc.vector.tensor_mul(
                    h_hat[:, t * NH : (t + 1) * NH],
                    y_sb[1][:, t * NH : (t + 1) * NH],
                    bc,
                )
                yield
            out["y"] = y_sb
            out["h"] = h_hat

        def fin_work(b, st):
            """Generator: T-layout partial sums for sample b (col sums +
            diag); runs inside iteration b+1's exp window."""
            csum, t_ = st["cs"], st["t"]
            ct16 = pp_T.tile([128, 16], F32, tag="T", name=f"ct16_{b}")
            for k0 in range(0, NCHUNK, 4):
                for k in range(k0, k0 + 4):
                    nc.tensor.matmul(
                        ct16[:, 8 + k : 9 + k],
                        t_[:, 128 * k : 128 * (k + 1)],
                        ones_k,
                        start=True,
                        stop=True,
                    )
                yield
            for k0 in range(0, NCHUNK, 4):
                for k in range(k0, k0 + 4):
                    nc.tensor.matmul(
                        ct16[:, k : k + 1],
                        csum[:, 128 * k : 128 * (k + 1)],
                        ones_k,
                        start=True,
                        stop=True,
                    )
                yield
            base = NCHUNK * (BPC + b)
            nc.vector.tensor_copy(OUT[:, base : base + NCHUNK], ct16[:, 0:8])
            dg = p_sm.tile([128, NCHUNK], F32, tag="dg")
            nc.vector.tensor_mul(
                dg, ct16[:, 8:16], invTH[:, 16 * b : 16 * b + 8]
            )
            nc.vector.tensor_reduce(
                out=OUT[:, 2 * NCHUNK * BPC + b : 2 * NCHUNK * BPC + b + 1],
                in_=dg,
                axis=mybir.AxisListType.X,
                op=ALU.add,
            )

        def drain(gens, n=1):
            """Advance each live generator up to n quanta."""
            for g in list(gens):
                for _ in range(n):
                    try:
                        next(g)
                    except StopIteration:
                        gens.remove(g)
                        break

        def emit_L(b, k, st):
            L = pp_L.tile([128, NPAT], F32, tag="L", name=f"L_{b}_{k}")
            ya, h_hat = st["y"][0], st["h"]
            for j in range(2):
                nc.tensor.matmul(
                    L[:, j * NH : (j + 1) * NH],
                    ya[:, 128 * k : 128 * (k + 1)],
                    h_hat[:, j * NH : (j + 1) * NH],
                    start=True,
                    stop=True,
                )
            return L

        def consume(b, st, side, carried, nxt):
            """Logits + exp + csum for sample b, interleaving side work in
            the exp gaps. Side work is drained 2x in the first chunks so
            tail(b+1) is done in time to pre-emit the next sample's first
            L at chunk 7 (no ACT gap across the sample seam)."""
            t_ = p_sq.tile([128, NPAT], BF16, tag="sq")
            nc.vector.tensor_mul(t_, st["y"][0], st["h"])

            csum = p_cs.tile([128, NPAT], BF16, tag="cs")
            Es = {}
            carried_next = {}
            for k in range(NCHUNK):
                L = carried.pop(k, None)
                if L is None:
                    L = emit_L(b, k, st)
                E = p_E.tile([128, NPAT], BF16, tag="E", name=f"E_{b}_{k}")
                Es[k] = E
                nc.scalar.activation(
                    out=E,
                    in_=L,
                    func=AF.Exp,
                    scale=invTH[:, 16 * b + k : 16 * b + k + 1],
                    accum_out=OUT[:, NCHUNK * b + k : NCHUNK * b + k + 1],
                )
                if k == NCHUNK - 1 and nxt is not None and "h" in nxt:
                    carried_next[0] = emit_L(b + 1, 0, nxt)
                if k == 1:
                    nc.vector.tensor_add(csum, Es[0], Es[1])
                elif k > 1:
                    nc.vector.tensor_add(csum, csum, E)
                drain(side, 2 if k < 4 else 1)
            return {"cs": csum, "t": t_}, carried_next

        # ACT-paced interleaved pipeline: conv DMA issued one sample ahead
        # of conv compute; conv compute + tail for b+1 drain inside
        # consume(b)'s exp gaps; fin(b-1) likewise.
        outs = {b: {} for b in range(BPC)}
        tout = {b: {} for b in range(BPC)}
        img_tiles = {0: conv_dma(0), 1: conv_dma(1)}
        for _ in conv_compute(0, img_tiles[0], outs[0]):
            pass
        for _ in tail_work(0, outs[0], tout[0]):
            pass
        fin_prev = None
        carried = {}
        for b in range(BPC):
            if b + 2 < BPC:
                img_tiles[b + 2] = conv_dma(b + 2)
            side = []
            if b + 1 < BPC:
                side.append(
                    chain(
                        conv_compute(b + 1, img_tiles[b + 1], outs[b + 1]),
                        tail_work(b + 1, outs[b + 1], tout[b + 1]),
                    )
                )
            if fin_prev is not None:
                side.append(fin_prev)
            nxt = tout[b + 1] if b + 1 < BPC else None
            st, carried = consume(b, tout[b], side, carried, nxt)
            drain(side, 100)  # finish any leftovers
            fin_prev = fin_work(b, st)
        for _ in fin_prev:
            pass

        nc.sync.dma_start(out=out_d[:], in_=OUT)

    nc.compile()
    return nc


_PROGRAM_CACHE: dict = {}


def _get_program(ln_s: float) -> bass.Bass:
    key = round(float(ln_s), 9)
    if key not in _PROGRAM_CACHE:
        _PROGRAM_CACHE[key] = build_program(float(ln_s))
    return _PROGRAM_CACHE[key]


def make_in_maps(imgs, hha, w1, b1, w2, b2):
    """Shard full inputs into per-core input maps (host-side, cheap)."""
    import ml_dtypes

    bf16 = ml_dtypes.bfloat16
    fp8 = ml_dtypes.float8_e4m3

    def prep_w(w):
        # [D,C,P,P] -> [(c,di,dj)=1024, D] -> [feat%128, chunk, ko, D] fp8 x64
        wf = np.transpose(np.asarray(w), (1, 2, 3, 0)).reshape(C * NOFF, D)
        wf = np.clip(wf * WSCALE, -240.0, 240.0)
        return np.ascontiguousarray(
            wf.reshape(NKC2, 2, 128, D).transpose(2, 0, 1, 3)
        ).astype(fp8)

    def prep_x(x):
        # stride==kernel -> im2col is a permutation:
        # [B,C,H,W] -> [B, (c,di,dj)=1024, (i,j)=1024] -> [B,128,NKC2,2,NPAT]
        B = x.shape[0]
        xp = np.asarray(x).reshape(B, C, H // P, P, W // P, P)
        xp = xp.transpose(0, 1, 3, 5, 2, 4).reshape(B, C * NOFF, NPAT)
        xp = np.clip(xp, -240.0, 240.0)
        return np.ascontiguousarray(
            xp.reshape(B, NKC2, 2, 128, NPAT).transpose(0, 3, 1, 2, 4)
        ).astype(fp8)

    w1t = prep_w(w1)
    w2t = prep_w(w2)
    imgs = prep_x(imgs)
    hha = prep_x(hha)
    b1 = np.ascontiguousarray(np.asarray(b1) * WSCALE, dtype=np.float32)
    b2 = np.ascontiguousarray(np.asarray(b2) * WSCALE, dtype=np.float32)
    ident = np.eye(128, dtype=bf16)
    sel = np.zeros((8, NCHUNK * 128), dtype=bf16)
    for q in range(NCHUNK):
        sel[q, 128 * q : 128 * (q + 1)] = 1.0
    maps = []
    for i in range(N_CORES):
        maps.append(
            {
                "imgs": np.ascontiguousarray(imgs[i * BPC : (i + 1) * BPC]),
                "hha": np.ascontiguousarray(hha[i * BPC : (i + 1) * BPC]),
                "w1t": w1t,
                "w2t": w2t,
                "b1": b1,
                "b2": b2,
                "ident": ident,
                "sel": sel,
            }
        )
    return maps


def combine_outputs(outs) -> np.float32:
    """Reduce the 8 per-core [128, 68] partial blocks to the scalar loss.
    Cols 0:64 are raw row/col exp-sums (host takes log in f64); cols 64:68
    are per-sample diag partial sums."""
    tot = np.float64(0.0)
    for o in outs:
        o = np.asarray(o, dtype=np.float64)
        lse_rc = np.log(o[:, : 2 * NCHUNK * BPC]).sum()
        diag = o[:, 2 * NCHUNK * BPC :].sum()
        tot += 0.5 * lse_rc - diag
    return np.float32(tot / (B_FULL * NPAT))


def run_spmd(imgs, hha, w1, b1, w2, b2, logit_scale, **kwargs):
    """Run on the 8 cores; returns (loss, BassKernelResults)."""
    ln_s = float(np.asarray(logit_scale))
    nc = _get_program(ln_s)
    in_maps = make_in_maps(imgs, hha, w1, b1, w2, b2)
    res = run_bass_kernel_spmd(nc, in_maps, list(range(N_CORES)), **kwargs)
    return combine_outputs([r["out"] for r in res.results]), res


def kernel(imgs, hha, w1, b1, w2, b2, logit_scale):
    loss, _ = run_spmd(imgs, hha, w1, b1, w2, b2, logit_scale)
    if not np.isfinite(loss):  # one-shot retry on a transient device glitch
        loss, _ = run_spmd(imgs, hha, w1, b1, w2, b2, logit_scale)
    return loss


if __name__ == "__main__":
    # smoke test against a tiny numpy reference of the math
    rng = np.random.default_rng(0)
    imgs = rng.standard_normal((B_FULL, C, H, W), dtype=np.float32)
    hha = rng.standard_normal((B_FULL, C, H, W), dtype=np.float32)
    w1 = rng.standard_normal((D, C, P, P), dtype=np.float32) * 0.03
    w2 = rng.standard_normal((D, C, P, P), dtype=np.float32) * 0.03
    b1 = np.zeros(D, np.float32)
    b2 = np.zeros(D, np.float32)
    ls = np.float32(np.log(1.0 / 0.07))
    print(kernel(imgs, hha, w1, b1, w2, b2, ls))
